# revision 50
# baseline (speedup 1.0000x reference)
"""GNN message-passing (graph convolution) kernel for 8 Trainium2 NeuronCores.

    out = relu(segment_sum(h[col], row) + bias),  h = x @ W

Strategy (dst-block sharding -- no collectives needed):
  * Host sorts edges by destination node and buckets them into 157 blocks of
    128 dst nodes.  Blocks are sorted by edge count and dealt snake-wise into
    20 slots x 8 cores so that slot s holds 8 similarly-sized blocks; the
    per-slot chunk count pb_s = max ceil(cnt/128) over its blocks is a program
    constant shared by all cores (SPMD), minimizing chunk padding.  Within a
    slot, edges are sorted by source node so early gathers depend only on a
    prefix of h.
  * Phase A (per core, replicated): h = x @ W on the PE in fp16 (PSUM fp32
    accumulate).  Nodes are pair-interleaved (partition p holds nodes 2p,
    2p+1 of each 256-node pair-tile) so h rows land adjacent in SBUF free
    dim and the DRAM h store runs with 512B descriptors (full DMA rate).
    x is shipped pre-transposed [kk, pt, k, e, p] and loads in 16 large DMAs
    through an 8-deep SBUF ring; PSUM->SBUF copies alternate DVE/ACT; the
    ACT engine's HWDGE queue stores h to DRAM in 10 batched writes, each
    unlocking more of the col-sorted gather stream.
  * Phase B: SWDGE dma_gather (HBM source) fetches 1024 edge rows per gather
    into val [128e, 8, 128f] slabs (4-deep ring); the DVE builds one-hot
    tiles S[e,n] = (iota == rowloc) (16-deep ring); the PE accumulates
    out_slot += S^T @ val over all chunks of the slot in PSUM fp32 -- an
    exact segment-sum.  The bias is folded in as one extra matmul per slot
    with constant operands (identity x bias-broadcast): no gather, no DVE.
    Gather idx tables are wrapped [16, n] and replicated only x2 (the SWDGE
    Q7 pair reads partitions 0-31; the rest is memset to 0 for the sim).
  * ACT applies ReLU PSUM->SBUF fp16; output stores are partition-major
    (512B runs, two slots per DMA).  The host scatters block rows back.

Numerics: fp16 operands with fp32 accumulation everywhere; the one-hot
matmul is exact, so the only error is fp16 rounding of x, W, h and the
output (~5e-4 relative).
"""

import sys

import numpy as np

sys.path.insert(0, "/opt/trn_rl_repo")

import concourse.bacc as bacc  # noqa: E402
import concourse.mybir as mybir  # noqa: E402
from concourse.bass_utils import run_bass_kernel_spmd  # noqa: E402

N_NODES = 20000
FIN = 256
FOUT = 128
N_EDGES = 640000

NTP = 158                # node tiles of 128 (padded even) -- h rows 20224
NPT = NTP // 2           # pair-tiles of 256 nodes: partition p holds 2p, 2p+1
NPAD = NTP * 128
NBLK = 157               # dst blocks of 128 nodes
NCORES = 8
NSLOT = 20               # block slots per core (slot 19: 5 real + 3 dummy)
NIDX = 1024              # idxs per dma_gather (8 chunks)
CPG = NIDX // 128        # chunks per gather
SCRATCH = 16384          # stock SWDGE ring (1024 descriptors)

FP16 = mybir.dt.float16
FP32 = mybir.dt.float32
I16 = mybir.dt.int16
I8 = mybir.dt.int8


def _plan(edge_index):
    """Sort/bucket edges; derive the SPMD-uniform slot structure."""
    row = np.asarray(edge_index[0]).astype(np.int64)
    col = np.asarray(edge_index[1]).astype(np.int64)
    order = np.argsort(row, kind="stable")
    rs = row[order].astype(np.int32)
    cs = col[order].astype(np.int32)

    blk = rs >> 7
    counts = np.bincount(blk, minlength=NBLK)
    starts = np.concatenate([[0], np.cumsum(counts)])

    big_first = np.argsort(counts, kind="stable")[::-1]  # block ids by size desc
    pbs = []
    slot_block = np.full((NCORES, NSLOT), -1, np.int64)
    for s in range(NSLOT):
        grp = big_first[s * NCORES:(s + 1) * NCORES]
        pbs.append(int(((counts[grp] + 127) // 128).max()))
        for c, b in enumerate(grp):
            slot_block[c, s] = b
    cum = np.concatenate([[0], np.cumsum(pbs)])
    nch = int(cum[-1])
    cpg = NIDX // 128
    ng = (nch + cpg - 1) // cpg
    nchp = ng * cpg
    return rs, cs, starts, slot_block, pbs, cum, nch, nchp, ng


def _host_prep(x, edge_index, weight, bias):
    """Cast/retile operands; build per-core gather index / rowloc tables."""
    x = np.asarray(x, np.float32)
    weight = np.asarray(weight, np.float32)
    bias = np.asarray(bias, np.float32)

    rs, cs, starts, slot_block, pbs, cum, nch, nchp, ng = _plan(edge_index)

    xpad = np.zeros((NPAD, FIN), np.float32)
    xpad[:N_NODES] = x
    # pair-tile layout: node pt*256 + 2p + e lives on partition p, so h rows
    # 2p, 2p+1 are adjacent in SBUF free dim -> 512B h-store descriptors.
    # xt[kk, pt*512 + k*256 + e*128 + p] = x[pt*256 + 2p + e, k*128 + kk]
    xt = np.ascontiguousarray(
        xpad.reshape(NPT, 128, 2, 2, 128)     # [pt, p, e, k, kk]
        .transpose(4, 0, 3, 2, 1)              # [kk, pt, k, e, p]
        .reshape(128, NPT * 512)
        .astype(np.float16)
    )
    # cst[:, 0:2, :] = W chunks; 2: iota; 3: identity; 4: bias broadcast
    cst = np.zeros((128, 5, 128), np.float16)
    cst[:, 0:2, :] = weight.reshape(2, 128, 128).transpose(1, 0, 2)
    cst[:, 2, :] = np.arange(128, dtype=np.float16)[None, :]
    cst[:, 3, :] = np.eye(128, dtype=np.float16)
    cst[:, 4, :] = bias.astype(np.float16)[None, :]

    gmax = np.zeros(ng, np.int64)
    col16 = np.zeros((NCORES, 32, ng * (NIDX // 16)), np.int16)
    rl8 = np.full((NCORES, 128, nchp), -1, np.int8)
    for c in range(NCORES):
        lin_col = np.zeros(nchp * 128, np.int32)
        lin_rl = np.full(nchp * 128, -1.0, np.float32)
        for s in range(NSLOT):
            b = slot_block[c, s]
            if b < 0:
                continue
            e0, e1 = int(starts[b]), int(starts[b + 1])
            k = e1 - e0
            j0 = int(cum[s]) * 128
            o = np.argsort(cs[e0:e1], kind="stable")
            lin_col[j0:j0 + k] = cs[e0:e1][o]
            lin_rl[j0:j0 + k] = (rs[e0:e1] - b * 128)[o]
        # SWDGE idx layout: idx i -> partition i%16, column i//16 (x8 repl.)
        col16[c] = np.tile(
            lin_col.reshape(nchp * 128 // 16, 16).T.astype(np.int16), (2, 1)
        )
        rl8[c] = lin_rl.reshape(nchp, 128).T.astype(np.int8)
        gmax = np.maximum(gmax, lin_col.reshape(ng, NIDX).max(axis=1))

    # per-gather h-frontier gate: h DRAM stores of 16 tiles (2048 rows)
    gates = [int(v) for v in (gmax // 2048 + 1)]
    meta = dict(
        pbs=pbs, cum=[int(v) for v in cum], nch=nch, nchp=nchp, ng=ng,
        gates=gates,
    )
    common = {"xt": xt, "cst": cst}
    per_core = [
        {"col": np.ascontiguousarray(col16[c]), "rl": np.ascontiguousarray(rl8[c])}
        for c in range(NCORES)
    ]
    return common, per_core, slot_block, meta


def _build_program(meta):
    pbs, cum = meta["pbs"], meta["cum"]
    nch, nchp, ng = meta["nch"], meta["nchp"], meta["ng"]
    gates = meta["gates"]
    chunk_slot = []                  # chunk j -> (slot, c)
    for s in range(NSLOT):
        for c in range(pbs[s]):
            chunk_slot.append((s, c))

    # cumulative segsum+bias matmul count after chunk j
    mm_after = []
    tot = 0
    for j in range(nch):
        s, c = chunk_slot[j]
        tot += 1
        if c == pbs[s] - 1:
            tot += 1
        mm_after.append(tot)

    NST = (NTP * 128 + 2047) // 2048  # h DRAM stores (16 tiles each)

    nc = bacc.Bacc("TRN2", dynamic_dma_scratch_size=SCRATCH)

    xt_d = nc.dram_tensor("xt", [128, NPT * 512], FP16, kind="ExternalInput")
    cst_d = nc.dram_tensor("cst", [128, 5, 128], FP16, kind="ExternalInput")
    col_d = nc.dram_tensor("col", [32, ng * (NIDX // 16)], I16, kind="ExternalInput")
    rl_d = nc.dram_tensor("rl", [128, nchp], I8, kind="ExternalInput")
    h_d = nc.dram_tensor("hbuf", [NTP * 128, 128], FP16)
    o_d = nc.dram_tensor("out", [128, NSLOT * 128], FP16, kind="ExternalOutput")

    from contextlib import ExitStack

    with ExitStack() as es:
        pha = [es.enter_context(nc.psum_tensor(f"pha{k}", [128, 512], FP32)) for k in range(4)]
        po = [es.enter_context(nc.psum_tensor(f"po{k}", [128, 512], FP32)) for k in range(4)]
        xt_sb = es.enter_context(nc.sbuf_tensor("xt_sb", [128, 8, 5, 2, 2, 128], FP16))
        cst_sb = es.enter_context(nc.sbuf_tensor("cst_sb", [128, 5, 128], FP16))
        h_sb = es.enter_context(nc.sbuf_tensor("h_sb", [128, NTP * 128], FP16))
        val_eb = es.enter_context(nc.sbuf_tensor("val_eb", [128, 8, CPG, 128], FP16))
        s_sb = es.enter_context(nc.sbuf_tensor("s_sb", [128, 16, 128], FP16))
        o_sb = es.enter_context(nc.sbuf_tensor("o_sb", [128, 2, 128], FP16))
        col_sb = es.enter_context(nc.sbuf_tensor("col_sb", [128, ng * (NIDX // 16)], I16))
        rl8_sb = es.enter_context(nc.sbuf_tensor("rl8_sb", [128, nchp], I8))
        rl_sb = es.enter_context(nc.sbuf_tensor("rl_sb", [128, nchp], FP32))

        s_x = [es.enter_context(nc.semaphore(f"s_x{k}")) for k in range(8)]
        s_ld = es.enter_context(nc.semaphore("s_ld"))
        s_msk = es.enter_context(nc.semaphore("s_msk"))
        s_hmm = es.enter_context(nc.semaphore("s_hmm"))
        s_hcp = es.enter_context(nc.semaphore("s_hcp"))
        s_hst = [es.enter_context(nc.semaphore(f"s_hst{k}")) for k in range(4)]
        s_gat = [es.enter_context(nc.semaphore(f"s_gat{k}")) for k in range(8)]
        s_s = es.enter_context(nc.semaphore("s_s"))
        s_prep = es.enter_context(nc.semaphore("s_prep"))
        s_cvt = es.enter_context(nc.semaphore("s_cvt"))
        s_smm = es.enter_context(nc.semaphore("s_smm"))
        s_act = es.enter_context(nc.semaphore("s_act"))
        s_ost = [es.enter_context(nc.semaphore(f"s_ost{k}")) for k in range(2)]
        block = es.enter_context(nc.Block())

        @block.sync
        def _(sync):
            sync.dma_start(cst_sb[:, :, :], cst_d[:, :, :]).then_inc(s_ld, 16)
            sync.dma_start(col_sb[0:32, :], col_d[:, :]).then_inc(s_ld, 16)
            sync.dma_start(rl8_sb[:, :], rl_d[:, :]).then_inc(s_ld, 16)
            for L in range(16):
                if L >= 8:
                    sync.wait_ge(s_hmm, 10 * (L - 7))
                npt = min(5, NPT - 5 * L)
                sync.dma_start(
                    xt_sb[:, L % 8, 0:npt, :, :, :],
                    xt_d[:, L * 2560:L * 2560 + npt * 512],
                ).then_inc(s_x[L % 8], 16)
            for k in range(NSLOT // 2):
                sync.wait_ge(s_act, 2 * (k + 1))
                if k >= 2:
                    sync.wait_ge(s_ost[k % 2], 16 * (k // 2))
                sync.dma_start(
                    o_d[:, k * 256:(k + 1) * 256], o_sb[:, :, :]
                ).then_inc(s_ost[k % 2], 16)

        @block.gpsimd
        def _(gpsimd):
            gpsimd.wait_ge(s_ld, 48)
            gpsimd.wait_ge(s_msk, 2)
            g_star = next(
                (g for g in range(ng) if gates[g] >= NST), ng
            )
            for g in range(ng):
                st = gates[g]
                prep = g == g_star
                if prep:
                    # generate descriptors BEFORE the final h-store gate so
                    # the SWDGE gen overlaps the preceding transfer
                    nix = min(NIDX, (nch - CPG * g) * 128)
                    gpsimd.dma_gather(
                        val_eb[:, g % 8, 0:nix // 128, :],
                        h_d[0:min(gates[g] * 2048, NTP * 128), :],
                        col_sb[:, g * (NIDX // 16):g * (NIDX // 16) + nix // 16],
                        nix,
                        nix,
                        128,
                        prepare_only=True,
                        sem=s_gat[g % 8],
                    ).then_inc(s_prep, 1)
                    gpsimd.wait_ge(s_prep, 1)
                for p in range(4):
                    cnt = len([k for k in range(st) if k % 4 == p])
                    if cnt:
                        gpsimd.wait_ge(s_hst[p], 16 * cnt)
                if g >= 8:
                    gpsimd.wait_ge(s_smm, mm_after[CPG * (g - 7) - 1])
                if prep:
                    gpsimd.trigger_dma(count=1)
                    continue
                # last gather: only its real chunks
                nix = min(NIDX, (nch - CPG * g) * 128)
                gpsimd.dma_gather(
                    val_eb[:, g % 8, 0:nix // 128, :],
                    h_d[0:min(gates[g] * 2048, NTP * 128), :],
                    col_sb[:, g * (NIDX // 16):g * (NIDX // 16) + nix // 16],
                    nix,
                    nix,
                    128,
                ).then_inc(s_gat[g % 8], 16)

        @block.tensor
        def _(tensor):
            tensor.wait_ge(s_ld, 48)
            # phase A: two pair-tiles (512 nodes) per PSUM bank
            for pt in range(NPT):
                L = pt // 5
                if pt % 5 == 0:
                    tensor.wait_ge(s_x[L % 8], 16 * (L // 8 + 1))
                b = pt // 2
                if pt % 2 == 0 and b >= 4:
                    tensor.wait_ge(s_hcp, b - 3)
                for e in range(2):
                    col = (pt % 2) * 256 + e * 128
                    tensor.matmul(
                        pha[b % 4][:, col:col + 128],
                        xt_sb[:, L % 8, pt % 5, 0, e, :],
                        cst_sb[:, 0, :],
                        start=True, stop=False,
                    )
                    tensor.matmul(
                        pha[b % 4][:, col:col + 128],
                        xt_sb[:, L % 8, pt % 5, 1, e, :],
                        cst_sb[:, 1, :],
                        start=False, stop=True,
                    ).then_inc(s_hmm, 1)
            # phase B: segment-sum straight off each gathered slab
            for k in range(ng):
                tensor.wait_ge(s_gat[k % 8], 16 * (k // 8 + 1))
                for jj in range(CPG * k, CPG * k + CPG):
                    if jj >= nch:
                        break
                    s, c = chunk_slot[jj]
                    tensor.wait_ge(s_s, jj + 1)
                    if c == 0 and s >= 4:
                        tensor.wait_ge(s_act, s - 3)
                    tensor.matmul(
                        po[s % 4][:, 0:128],
                        s_sb[:, jj % 16, :],
                        val_eb[:, k % 8, jj % CPG, :],
                        start=(c == 0), stop=False,
                    ).then_inc(s_smm, 1)
                    if c == pbs[s] - 1:
                        tensor.matmul(
                            po[s % 4][:, 0:128],
                            cst_sb[:, 3, :],
                            cst_sb[:, 4, :],
                            start=False, stop=True,
                        ).then_inc(s_smm, 1)

        @block.vector
        def _(vector):
            # top idx partitions are never read by SWDGE; zero them so the
            # interp's bounds assert sees valid values
            vector.memset(col_sb[32:64, :], 0).then_inc(s_msk, 1)
            vector.memset(col_sb[64:128, :], 0).then_inc(s_msk, 1)
            vector.wait_ge(s_ld, 48)
            # phase A: PSUM fp32 -> SBUF fp16, 2 pair-tiles per copy
            for b in range((NPT + 1) // 2):
                npt = min(2, NPT - 2 * b)
                vector.wait_ge(s_hmm, 4 * b + 2 * npt)
                vector.tensor_copy(
                    h_sb[:, b * 512:b * 512 + npt * 256],
                    pha[b % 4][:, 0:npt * 256],
                ).then_inc(s_hcp, 1)
            # phase B: widen rowloc int8 -> fp32, then one-hot tiles
            vector.tensor_copy(rl_sb[:, :], rl8_sb[:, :]).then_inc(s_cvt, 1)
            vector.wait_ge(s_cvt, 1)
            for j in range(nch):
                if j >= 16:
                    vector.wait_ge(s_smm, mm_after[j - 16])
                vector.tensor_scalar(
                    s_sb[:, j % 16, :],
                    cst_sb[:, 2, :],
                    rl_sb[:, j:j + 1],
                    None,
                    mybir.AluOpType.is_equal,
                ).then_inc(s_s, 1)

        @block.scalar
        def _(scalar):
            # h DRAM stores on the otherwise-idle ACT hwdge queue
            for k in range(NST):
                rows = min(2048, NTP * 128 - k * 2048)
                scalar.wait_ge(s_hcp, min(4 * (k + 1), (NPT + 1) // 2))
                if k >= 2:
                    scalar.wait_ge(s_hst[k % 2], 16 * (k // 2))
                scalar.dma_start(
                    h_d[k * 2048:k * 2048 + rows, :].rearrange(
                        "(t p e) f -> p t (e f)", p=128, e=2
                    ),
                    h_sb[:, k * 2048:k * 2048 + rows],
                ).then_inc(s_hst[k % 2], 16)
            for s in range(NSLOT):
                scalar.wait_ge(s_smm, mm_after[cum[s + 1] - 1])
                if s >= 2:
                    # o_sb slot s%2 (written by relu s-2) is read by store (s-2)//2
                    k0 = (s - 2) // 2
                    scalar.wait_ge(s_ost[k0 % 2], 16 * (k0 // 2 + 1))
                scalar.activation(
                    o_sb[:, s % 2, :], po[s % 4][:, 0:128],
                    mybir.ActivationFunctionType.Relu,
                ).then_inc(s_act, 1)

    nc.compile()
    return nc


def _decode_out(oc):
    """[128, NSLOT*128] partition-major -> [NSLOT*128 rows, 128] fp32."""
    return np.ascontiguousarray(
        oc.reshape(128, NSLOT, 128).transpose(1, 0, 2).reshape(NSLOT * 128, 128)
    ).astype(np.float32)


def _run(x, edge_index, weight, bias, trace=False):
    common, per_core, slot_block, meta = _host_prep(x, edge_index, weight, bias)
    nc = _build_program(meta)
    in_maps = [dict(common, **per_core[c]) for c in range(NCORES)]
    res = run_bass_kernel_spmd(nc, in_maps, list(range(NCORES)), trace=trace)
    out = np.zeros((NBLK * 128, FOUT), np.float32)
    for c in range(NCORES):
        oc = _decode_out(np.asarray(res.results[c]["out"]))
        for s in range(NSLOT):
            b = slot_block[c, s]
            if b >= 0:
                out[b * 128:(b + 1) * 128] = oc[s * 128:(s + 1) * 128]
    return np.ascontiguousarray(out[:N_NODES]), res


def kernel(x, edge_index, weight, bias):
    out, _ = _run(x, edge_index, weight, bias, trace=False)
    return out


# revision 53
# speedup vs baseline: 1.0541x; 1.0541x over previous
"""GNN message-passing (graph convolution) kernel for 8 Trainium2 NeuronCores.

    out = relu(segment_sum(h[col], row) + bias),  h = x @ W

Strategy (dst-block sharding -- no collectives needed):
  * Host sorts edges by destination node and buckets them into 157 blocks of
    128 dst nodes.  Blocks are sorted by edge count and dealt snake-wise into
    20 slots x 8 cores so that slot s holds 8 similarly-sized blocks; the
    per-slot chunk count pb_s = max ceil(cnt/128) over its blocks is a program
    constant shared by all cores (SPMD), minimizing chunk padding.  Within a
    slot, edges are sorted by source node so early gathers depend only on a
    prefix of h.
  * Phase A (per core, replicated): h = x @ W on the PE in fp16 (PSUM fp32
    accumulate).  Nodes are pair-interleaved (partition p holds nodes 2p,
    2p+1 of each 256-node pair-tile) so h rows land adjacent in SBUF free
    dim and the DRAM h store runs with 512B descriptors (full DMA rate).
    x is shipped pre-transposed [kk, pt, k, e, p] and loads in 16 large DMAs
    through an 8-deep SBUF ring; PSUM->SBUF copies alternate DVE/ACT; the
    ACT engine's HWDGE queue stores h to DRAM in 10 batched writes, each
    unlocking more of the col-sorted gather stream.
  * Phase B: SWDGE dma_gather (HBM source) fetches 1024 edge rows per gather
    into val [128e, 8, 128f] slabs (4-deep ring); the DVE builds one-hot
    tiles S[e,n] = (iota == rowloc) (16-deep ring); the PE accumulates
    out_slot += S^T @ val over all chunks of the slot in PSUM fp32 -- an
    exact segment-sum.  The bias is folded in as one extra matmul per slot
    with constant operands (identity x bias-broadcast): no gather, no DVE.
    Gather idx tables are wrapped [16, n] and replicated only x2 (the SWDGE
    Q7 pair reads partitions 0-31; the rest is memset to 0 for the sim).
  * ACT applies ReLU PSUM->SBUF fp16; output stores are partition-major
    (512B runs, two slots per DMA).  The host scatters block rows back.

Numerics: fp16 operands with fp32 accumulation everywhere; the one-hot
matmul is exact, so the only error is fp16 rounding of x, W, h and the
output (~5e-4 relative).
"""

import sys

import numpy as np

sys.path.insert(0, "/opt/trn_rl_repo")

import concourse.bacc as bacc  # noqa: E402
import concourse.mybir as mybir  # noqa: E402
from concourse.bass_utils import run_bass_kernel_spmd  # noqa: E402

N_NODES = 20000
FIN = 256
FOUT = 128
N_EDGES = 640000

NTP = 158                # node tiles of 128 (padded even) -- h rows 20224
NPT = NTP // 2           # pair-tiles of 256 nodes: partition p holds 2p, 2p+1
NPAD = NTP * 128
NBLK = 157               # dst blocks of 128 nodes
NCORES = 8
NSLOT = 20               # block slots per core (slot 19: 5 real + 3 dummy)
NIDX = 1024              # idxs per dma_gather (8 chunks)
CPG = NIDX // 128        # chunks per gather
SCRATCH = 16384          # stock SWDGE ring (1024 descriptors)

FP16 = mybir.dt.float16
FP32 = mybir.dt.float32
I16 = mybir.dt.int16
I8 = mybir.dt.int8


def _plan(edge_index):
    """Sort/bucket edges; derive the SPMD-uniform slot structure."""
    row = np.asarray(edge_index[0]).astype(np.int64)
    col = np.asarray(edge_index[1]).astype(np.int64)
    order = np.argsort(row, kind="stable")
    rs = row[order].astype(np.int32)
    cs = col[order].astype(np.int32)

    blk = rs >> 7
    counts = np.bincount(blk, minlength=NBLK)
    starts = np.concatenate([[0], np.cumsum(counts)])

    # lane packing: an even-col and an odd-col edge sharing a row pair
    # (2k, 2k+1) occupy ONE gather lane (512B descriptor covers both rows)
    nlanes = np.zeros(NBLK, np.int64)
    for b in range(NBLK):
        c = cs[starts[b]:starts[b + 1]]
        ne = np.bincount((c >> 1)[(c & 1) == 0], minlength=NPAD // 2)
        no = np.bincount((c >> 1)[(c & 1) == 1], minlength=NPAD // 2)
        nlanes[b] = int(np.maximum(ne, no).sum())

    big_first = np.argsort(counts, kind="stable")[::-1]  # block ids by size desc
    pbs = []
    slot_block = np.full((NCORES, NSLOT), -1, np.int64)
    for s in range(NSLOT):
        grp = big_first[s * NCORES:(s + 1) * NCORES]
        pbs.append(int(((nlanes[grp] + 127) // 128).max()))
        for c, b in enumerate(grp):
            slot_block[c, s] = b
    cum = np.concatenate([[0], np.cumsum(pbs)])
    nch = int(cum[-1])
    cpg = NIDX // 128
    ng = (nch + cpg - 1) // cpg
    nchp = ng * cpg
    return rs, cs, starts, slot_block, pbs, cum, nch, nchp, ng


def _host_prep(x, edge_index, weight, bias):
    """Cast/retile operands; build per-core gather index / rowloc tables."""
    x = np.asarray(x, np.float32)
    weight = np.asarray(weight, np.float32)
    bias = np.asarray(bias, np.float32)

    rs, cs, starts, slot_block, pbs, cum, nch, nchp, ng = _plan(edge_index)

    xpad = np.zeros((NPAD, FIN), np.float32)
    xpad[:N_NODES] = x
    # pair-tile layout: node pt*256 + 2p + e lives on partition p, so h rows
    # 2p, 2p+1 are adjacent in SBUF free dim -> 512B h-store descriptors.
    # xt[kk, pt*512 + k*256 + e*128 + p] = x[pt*256 + 2p + e, k*128 + kk]
    xt = np.ascontiguousarray(
        xpad.reshape(NPT, 128, 2, 2, 128)     # [pt, p, e, k, kk]
        .transpose(4, 0, 3, 2, 1)              # [kk, pt, k, e, p]
        .reshape(128, NPT * 512)
        .astype(np.float16)
    )
    # cst[:, 0:2, :] = W chunks; 2: iota; 3: identity; 4: bias broadcast
    cst = np.zeros((128, 5, 128), np.float16)
    cst[:, 0:2, :] = weight.reshape(2, 128, 128).transpose(1, 0, 2)
    cst[:, 2, :] = np.arange(128, dtype=np.float16)[None, :]
    cst[:, 3, :] = np.eye(128, dtype=np.float16)
    cst[:, 4, :] = bias.astype(np.float16)[None, :]

    gmax = np.zeros(ng, np.int64)
    col16 = np.zeros((NCORES, 32, ng * (NIDX // 16)), np.int16)
    rl8 = np.full((NCORES, 128, 2 * nchp), -1, np.int8)
    for c in range(NCORES):
        lin_col = np.zeros(nchp * 128, np.int32)
        lin_rl = np.full((nchp * 128, 2), -1.0, np.float32)
        for s in range(NSLOT):
            b = slot_block[c, s]
            if b < 0:
                continue
            e0, e1 = int(starts[b]), int(starts[b + 1])
            cc = cs[e0:e1]
            rr = rs[e0:e1] - b * 128
            j0 = int(cum[s]) * 128
            pv, par = cc >> 1, cc & 1
            # lane base per pair value + rank within (pair, parity)
            o = np.lexsort((par, pv))
            pvs, pars, rrs = pv[o], par[o], rr[o]
            ne = np.bincount(pvs[pars == 0], minlength=NPAD // 2)
            no = np.bincount(pvs[pars == 1], minlength=NPAD // 2)
            lanes_v = np.maximum(ne, no)
            base = np.zeros(NPAD // 2, np.int64)
            base[1:] = np.cumsum(lanes_v)[:-1]
            run = np.r_[True, (pvs[1:] != pvs[:-1]) | (pars[1:] != pars[:-1])]
            rstart = np.maximum.accumulate(np.where(run, np.arange(len(pvs)), 0))
            rank = np.arange(len(pvs)) - rstart
            lane = base[pvs] + rank
            lin_col[j0 + lane] = pvs
            lin_rl[j0 + lane, pars] = rrs
        # SWDGE idx layout: idx i -> partition i%16, column i//16 (x2 repl.)
        col16[c] = np.tile(
            lin_col.reshape(nchp * 128 // 16, 16).T.astype(np.int16), (2, 1)
        )
        rl8[c] = (
            lin_rl.reshape(nchp, 128, 2).transpose(1, 0, 2).reshape(128, 2 * nchp)
        ).astype(np.int8)
        gmax = np.maximum(gmax, lin_col.reshape(ng, NIDX).max(axis=1))

    # per-gather h-frontier gate: h stores of 1024 pair-rows each
    gates = [int(v) for v in (gmax // 1024 + 1)]
    meta = dict(
        pbs=pbs, cum=[int(v) for v in cum], nch=nch, nchp=nchp, ng=ng,
        gates=gates,
    )
    common = {"xt": xt, "cst": cst}
    per_core = [
        {"col": np.ascontiguousarray(col16[c]), "rl": np.ascontiguousarray(rl8[c])}
        for c in range(NCORES)
    ]
    return common, per_core, slot_block, meta


def _build_program(meta):
    pbs, cum = meta["pbs"], meta["cum"]
    nch, nchp, ng = meta["nch"], meta["nchp"], meta["ng"]
    gates = meta["gates"]
    chunk_slot = []                  # chunk j -> (slot, c)
    for s in range(NSLOT):
        for c in range(pbs[s]):
            chunk_slot.append((s, c))

    # cumulative segsum+bias matmul count after chunk j
    mm_after = []
    tot = 0
    for j in range(nch):
        s, c = chunk_slot[j]
        tot += 2
        if c == pbs[s] - 1:
            tot += 1
        mm_after.append(tot)

    NST = (NTP * 128 + 2047) // 2048  # h DRAM stores (16 tiles each)

    nc = bacc.Bacc("TRN2", dynamic_dma_scratch_size=SCRATCH)

    xt_d = nc.dram_tensor("xt", [128, NPT * 512], FP16, kind="ExternalInput")
    cst_d = nc.dram_tensor("cst", [128, 5, 128], FP16, kind="ExternalInput")
    col_d = nc.dram_tensor("col", [32, ng * (NIDX // 16)], I16, kind="ExternalInput")
    rl_d = nc.dram_tensor("rl", [128, 2 * nchp], I8, kind="ExternalInput")
    h_d = nc.dram_tensor("hbuf", [NTP * 64, 256], FP16)
    o_d = nc.dram_tensor("out", [128, NSLOT * 128], FP16, kind="ExternalOutput")

    from contextlib import ExitStack

    with ExitStack() as es:
        pha = [es.enter_context(nc.psum_tensor(f"pha{k}", [128, 512], FP32)) for k in range(4)]
        po = [es.enter_context(nc.psum_tensor(f"po{k}", [128, 512], FP32)) for k in range(4)]
        xt_sb = es.enter_context(nc.sbuf_tensor("xt_sb", [128, 8, 5, 2, 2, 128], FP16))
        cst_sb = es.enter_context(nc.sbuf_tensor("cst_sb", [128, 5, 128], FP16))
        h_sb = es.enter_context(nc.sbuf_tensor("h_sb", [128, NTP * 128], FP16))
        val_eb = es.enter_context(nc.sbuf_tensor("val_eb", [128, 8, CPG, 256], FP16))
        s_sb = es.enter_context(nc.sbuf_tensor("s_sb", [128, 32, 128], FP16))
        o_sb = es.enter_context(nc.sbuf_tensor("o_sb", [128, 2, 128], FP16))
        col_sb = es.enter_context(nc.sbuf_tensor("col_sb", [128, ng * (NIDX // 16)], I16))
        rl8_sb = es.enter_context(nc.sbuf_tensor("rl8_sb", [128, 2 * nchp], I8))
        rl_sb = es.enter_context(nc.sbuf_tensor("rl_sb", [128, 2 * nchp], FP32))

        s_x = [es.enter_context(nc.semaphore(f"s_x{k}")) for k in range(8)]
        s_ld = es.enter_context(nc.semaphore("s_ld"))
        s_msk = es.enter_context(nc.semaphore("s_msk"))
        s_hmm = es.enter_context(nc.semaphore("s_hmm"))
        s_hcp = es.enter_context(nc.semaphore("s_hcp"))
        s_hst = [es.enter_context(nc.semaphore(f"s_hst{k}")) for k in range(4)]
        s_gat = [es.enter_context(nc.semaphore(f"s_gat{k}")) for k in range(8)]
        s_s = es.enter_context(nc.semaphore("s_s"))
        s_prep = es.enter_context(nc.semaphore("s_prep"))
        s_cvt = es.enter_context(nc.semaphore("s_cvt"))
        s_smm = es.enter_context(nc.semaphore("s_smm"))
        s_act = es.enter_context(nc.semaphore("s_act"))
        s_ost = [es.enter_context(nc.semaphore(f"s_ost{k}")) for k in range(2)]
        block = es.enter_context(nc.Block())

        @block.sync
        def _(sync):
            sync.dma_start(cst_sb[:, :, :], cst_d[:, :, :]).then_inc(s_ld, 16)
            sync.dma_start(col_sb[0:32, :], col_d[:, :]).then_inc(s_ld, 16)
            sync.dma_start(rl8_sb[:, :], rl_d[:, :]).then_inc(s_ld, 16)
            for L in range(16):
                if L >= 8:
                    sync.wait_ge(s_hmm, 10 * (L - 7))
                npt = min(5, NPT - 5 * L)
                sync.dma_start(
                    xt_sb[:, L % 8, 0:npt, :, :, :],
                    xt_d[:, L * 2560:L * 2560 + npt * 512],
                ).then_inc(s_x[L % 8], 16)
            for k in range(NSLOT // 2):
                sync.wait_ge(s_act, 2 * (k + 1))
                if k >= 2:
                    sync.wait_ge(s_ost[k % 2], 16 * (k // 2))
                sync.dma_start(
                    o_d[:, k * 256:(k + 1) * 256], o_sb[:, :, :]
                ).then_inc(s_ost[k % 2], 16)

        @block.gpsimd
        def _(gpsimd):
            gpsimd.wait_ge(s_ld, 48)
            gpsimd.wait_ge(s_msk, 2)
            g_star = next(
                (g for g in range(ng) if gates[g] >= NST), ng
            )
            for g in range(ng):
                st = gates[g]
                prep = g == g_star
                if prep:
                    # generate descriptors BEFORE the final h-store gate so
                    # the SWDGE gen overlaps the preceding transfer
                    nix = min(NIDX, (nch - CPG * g) * 128)
                    gpsimd.dma_gather(
                        val_eb[:, g % 8, 0:nix // 128, :],
                        h_d[0:min(gates[g] * 1024, NTP * 64), :],
                        col_sb[:, g * (NIDX // 16):g * (NIDX // 16) + nix // 16],
                        nix,
                        nix,
                        256,
                        prepare_only=True,
                        sem=s_gat[g % 8],
                    ).then_inc(s_prep, 1)
                    gpsimd.wait_ge(s_prep, 1)
                for p in range(4):
                    cnt = len([k for k in range(st) if k % 4 == p])
                    if cnt:
                        gpsimd.wait_ge(s_hst[p], 16 * cnt)
                if g >= 8:
                    gpsimd.wait_ge(s_smm, mm_after[CPG * (g - 7) - 1])
                if prep:
                    gpsimd.trigger_dma(count=1)
                    continue
                # last gather: only its real chunks
                nix = min(NIDX, (nch - CPG * g) * 128)
                gpsimd.dma_gather(
                    val_eb[:, g % 8, 0:nix // 128, :],
                    h_d[0:min(gates[g] * 1024, NTP * 64), :],
                    col_sb[:, g * (NIDX // 16):g * (NIDX // 16) + nix // 16],
                    nix,
                    nix,
                    256,
                ).then_inc(s_gat[g % 8], 16)

        @block.tensor
        def _(tensor):
            tensor.wait_ge(s_ld, 48)
            # phase A: two pair-tiles (512 nodes) per PSUM bank
            for pt in range(NPT):
                L = pt // 5
                if pt % 5 == 0:
                    tensor.wait_ge(s_x[L % 8], 16 * (L // 8 + 1))
                b = pt // 2
                if pt % 2 == 0 and b >= 4:
                    tensor.wait_ge(s_hcp, b - 3)
                for e in range(2):
                    col = (pt % 2) * 256 + e * 128
                    tensor.matmul(
                        pha[b % 4][:, col:col + 128],
                        xt_sb[:, L % 8, pt % 5, 0, e, :],
                        cst_sb[:, 0, :],
                        start=True, stop=False,
                    )
                    tensor.matmul(
                        pha[b % 4][:, col:col + 128],
                        xt_sb[:, L % 8, pt % 5, 1, e, :],
                        cst_sb[:, 1, :],
                        start=False, stop=True,
                    ).then_inc(s_hmm, 1)
            # phase B: segment-sum straight off each gathered slab
            for k in range(ng):
                tensor.wait_ge(s_gat[k % 8], 16 * (k // 8 + 1))
                for jj in range(CPG * k, CPG * k + CPG):
                    if jj >= nch:
                        break
                    s, c = chunk_slot[jj]
                    tensor.wait_ge(s_s, 2 * jj + 2)
                    if c == 0 and s >= 4:
                        tensor.wait_ge(s_act, s - 3)
                    for t in range(2):
                        tensor.matmul(
                            po[s % 4][:, 0:128],
                            s_sb[:, (2 * jj + t) % 32, :],
                            val_eb[:, k % 8, jj % CPG, t * 128:(t + 1) * 128],
                            start=(c == 0 and t == 0), stop=False,
                        ).then_inc(s_smm, 1)
                    if c == pbs[s] - 1:
                        tensor.matmul(
                            po[s % 4][:, 0:128],
                            cst_sb[:, 3, :],
                            cst_sb[:, 4, :],
                            start=False, stop=True,
                        ).then_inc(s_smm, 1)

        @block.vector
        def _(vector):
            # top idx partitions are never read by SWDGE; zero them so the
            # interp's bounds assert sees valid values
            vector.memset(col_sb[32:64, :], 0).then_inc(s_msk, 1)
            vector.memset(col_sb[64:128, :], 0).then_inc(s_msk, 1)
            vector.wait_ge(s_ld, 48)
            # phase A: PSUM fp32 -> SBUF fp16, 2 pair-tiles per copy
            for b in range((NPT + 1) // 2):
                npt = min(2, NPT - 2 * b)
                vector.wait_ge(s_hmm, 4 * b + 2 * npt)
                vector.tensor_copy(
                    h_sb[:, b * 512:b * 512 + npt * 256],
                    pha[b % 4][:, 0:npt * 256],
                ).then_inc(s_hcp, 1)
            # phase B: widen rowloc int8 -> fp32, then one-hot tiles
            vector.tensor_copy(rl_sb[:, :], rl8_sb[:, :]).then_inc(s_cvt, 1)
            vector.wait_ge(s_cvt, 1)
            for j2 in range(2 * nch):
                if j2 >= 32:
                    vector.wait_ge(s_smm, mm_after[(j2 - 32) // 2])
                vector.tensor_scalar(
                    s_sb[:, j2 % 32, :],
                    cst_sb[:, 2, :],
                    rl_sb[:, j2:j2 + 1],
                    None,
                    mybir.AluOpType.is_equal,
                ).then_inc(s_s, 1)

        @block.scalar
        def _(scalar):
            # h DRAM stores on the otherwise-idle ACT hwdge queue
            for k in range(NST):
                rows = min(2048, NTP * 128 - k * 2048)
                scalar.wait_ge(s_hcp, min(4 * (k + 1), (NPT + 1) // 2))
                if k >= 2:
                    scalar.wait_ge(s_hst[k % 2], 16 * (k // 2))
                scalar.dma_start(
                    h_d[k * 2048:k * 2048 + rows, :].rearrange(
                        "(t p e) f -> p t (e f)", p=128, e=2
                    ),
                    h_sb[:, k * 2048:k * 2048 + rows],
                ).then_inc(s_hst[k % 2], 16)
            for s in range(NSLOT):
                scalar.wait_ge(s_smm, mm_after[cum[s + 1] - 1])
                if s >= 2:
                    # o_sb slot s%2 (written by relu s-2) is read by store (s-2)//2
                    k0 = (s - 2) // 2
                    scalar.wait_ge(s_ost[k0 % 2], 16 * (k0 // 2 + 1))
                scalar.activation(
                    o_sb[:, s % 2, :], po[s % 4][:, 0:128],
                    mybir.ActivationFunctionType.Relu,
                ).then_inc(s_act, 1)

    nc.compile()
    return nc


def _decode_out(oc):
    """[128, NSLOT*128] partition-major -> [NSLOT*128 rows, 128] fp32."""
    return np.ascontiguousarray(
        oc.reshape(128, NSLOT, 128).transpose(1, 0, 2).reshape(NSLOT * 128, 128)
    ).astype(np.float32)


def _run(x, edge_index, weight, bias, trace=False):
    common, per_core, slot_block, meta = _host_prep(x, edge_index, weight, bias)
    nc = _build_program(meta)
    in_maps = [dict(common, **per_core[c]) for c in range(NCORES)]
    res = run_bass_kernel_spmd(nc, in_maps, list(range(NCORES)), trace=trace)
    out = np.zeros((NBLK * 128, FOUT), np.float32)
    for c in range(NCORES):
        oc = _decode_out(np.asarray(res.results[c]["out"]))
        for s in range(NSLOT):
            b = slot_block[c, s]
            if b >= 0:
                out[b * 128:(b + 1) * 128] = oc[s * 128:(s + 1) * 128]
    return np.ascontiguousarray(out[:N_NODES]), res


def kernel(x, edge_index, weight, bias):
    out, _ = _run(x, edge_index, weight, bias, trace=False)
    return out
